# revision 7
# baseline (speedup 1.0000x reference)
"""Trainium2 kernel v2 for nn_B3SplineUWT: 3-level B3-spline UWT,
data-parallel over 8 NeuronCores, bf16 data path.

kernel(x: [8,1024,1024] f32) -> [8,4,1024,1024] f32  (w1,w2,w3,c3)

Per core, per level (chained c_{j+1} = A_w A_h c_j):
  - H-conv: PE banded matmuls, lhsT = A_d/16 blocks (bf16-exact).
  - W-conv, "ds" chunks: PE transpose -> evac -> data-stationary
    matmul (moving = 16*A_d blocks) -> ACT evac. No transpose-back.
  - W-conv, "route" chunks: 4 binomial (1,1) passes on DVE/Pool
    (taps 1,4,6,4,1 = (1,1)^4; /16 pre-folded into H consts).
  - w_j = prev - cur: bf16 TT; outputs DMA'd as bf16, host upcasts.
"""
import numpy as np
import ml_dtypes

import concourse.bacc as bacc
import concourse.bass as bass
import concourse.mybir as mybir
import concourse.tile as tile
from concourse.bass_utils import run_bass_kernel_spmd

F32 = mybir.dt.float32
BF16 = mybir.dt.bfloat16
ADD = mybir.AluOpType.add
SUB = mybir.AluOpType.subtract
COPY = mybir.ActivationFunctionType.Copy

B = 8
H = 1024
W = 1024
P = 128
NCH = H // P
LEVELS = 3
DILS = (1, 2, 4)
MARG = 8
WE = W + 2 * MARG

TAPS = {0: 3.0 / 8, 1: 1.0 / 4, 2: 1.0 / 16}

DEFAULT_CFG = {
    "ds": ((2,), (0,), (1,)),          # per-level ds chunk sets
    "si": ((6,), (3, 6), (3, 5, 7)),   # per-level shifted-identity chunks
    "hy": ((), (), ()),                # hybrid PE(1,2,1)+DVE(1,2,1) chunks
    "hy_marg_dve": False,              # hybrid p2 margins on DVE not Pool
    "tt_split": False,                 # tT evac: half ACT, half DVE
    "p1_dve": (False, False, False),   # per-level: p1 pass on DVE not Pool
    "alt_q": False,                    # alternate HWDGE queues SP/ACT
    "sub_pool": ((), (), ()),          # per-level chunks whose sub -> Pool
    "xq": 4,                           # number of input DMAs
    "x_act_q": False,                  # issue input DMAs from ACT queue
    "x_split": False,                  # input DMAs split across both rings
    "ph1024": False,                   # single 1024-wide H psum + one evac
    "qorder": True,                    # quarter-interleaved emission
    "p1k": None,                       # per-level: first-k route p1 on DVE
    "c3_act_q": False,                 # c3 DMAs on ACT HWDGE ring
    "x_first_small": True,             # input DMA sizes 1,1,2,4 chunks
    "warmup": 0,                       # dummy PE MMs after consts load
    "out8_l2": True,                   # per-chunk output DMAs on last level
    "c_act_q": False,                  # consts DMA on ACT HWDGE ring
    "halo": False,                     # DMA-gathered halo H-conv (16 MMs)
    "halo_split": 1,                   # pieces per halo DMA
}


def _reflect(i, n):
    if i < 0:
        return -i
    if i >= n:
        return 2 * (n - 1) - i
    return i


def _build_full(d, scale):
    full = np.zeros((H, H), np.float32)
    for r in range(H):
        for o in (-2 * d, -d, 0, d, 2 * d):
            full[_reflect(r + o, H), r] += TAPS[abs(o) // d] * scale
    return full


def _pack_consts(with_id2=False, with_halo=False):
    mats, seen = [], {}
    hidx, widx = [], []
    eidx = []   # per level: (interior16_off, co0_off, co7_off)

    def intern(blk):
        key = blk.tobytes()
        if key not in seen:
            seen[key] = len(mats) * P
            mats.append(blk)
        return seen[key]

    for d in DILS:
        fh = _build_full(d, 1.0 / 16)
        idx = {}
        for co in range(NCH):
            for ci in range(NCH):
                blk = np.ascontiguousarray(fh[ci * P:(ci + 1) * P,
                                              co * P:(co + 1) * P])
                if np.any(blk != 0):
                    idx[(co, ci)] = intern(blk.astype(ml_dtypes.bfloat16))
        hidx.append(idx)
    for d in (DILS if with_halo else ()):
        # halo stationaries: per chunk a [16,P] block (rows 0-7 =
        # bottom-halo taps from chunk co+1 rows 0-7; rows 8-15 = top-halo
        # taps from chunk co-1 rows 120-127), padded into [128,P] slots.
        # Chunk 0/7 get 8-row one-sided blocks.
        fh = _build_full(d, 1.0 / 16)
        lvl = {}
        for co in range(NCH):
            blk = np.zeros((P, P), np.float32)
            nr = 0
            if co < NCH - 1:
                blk[0:8, :] = fh[(co + 1) * P:(co + 1) * P + 8,
                                 co * P:(co + 1) * P]
                nr = 8
            if co > 0:
                if co == NCH - 1:
                    # top-only at rows 0-7 (read via hl column 8, base 0)
                    blk[0:8, :] = fh[co * P - 8:co * P, co * P:(co + 1) * P]
                    nr = 8
                else:
                    blk[8:16, :] = fh[co * P - 8:co * P,
                                      co * P:(co + 1) * P]
                    nr = 16
            lvl[co] = (intern(blk.astype(ml_dtypes.bfloat16)), nr)
        eidx.append(lvl)
    for d in DILS:
        fw = _build_full(d, 16.0)
        idx = {}
        for co in range(NCH):
            for ci in range(NCH):
                blk = np.ascontiguousarray(fw[ci * P:(ci + 1) * P,
                                              co * P:(co + 1) * P])
                if np.any(blk != 0):
                    idx[(co, ci)] = intern(blk.astype(ml_dtypes.bfloat16))
        widx.append(idx)
    ident_off = len(mats) * P
    mats.append(np.eye(P, dtype=ml_dtypes.bfloat16))
    id4_off = len(mats) * P
    mats.append((4 * np.eye(P)).astype(ml_dtypes.bfloat16))
    id6_off = len(mats) * P
    mats.append((6 * np.eye(P)).astype(ml_dtypes.bfloat16))
    id2_off = None
    if with_id2:
        id2_off = len(mats) * P
        mats.append((2 * np.eye(P)).astype(ml_dtypes.bfloat16))
    packed = np.ascontiguousarray(np.concatenate(mats, axis=1))
    return packed, hidx, widx, eidx, (ident_off, id4_off, id6_off, id2_off)


def _build_program(cfg=None):
    cfg = dict(DEFAULT_CFG, **(cfg or {}))
    consts_np, hidx, widx, eidx, (ident_off, id4_off, id6_off, id2_off) = \
        _pack_consts(with_id2=any(cfg["hy"]), with_halo=cfg["halo"])
    ncols = consts_np.shape[1]

    nc = bacc.Bacc("TRN2", target_bir_lowering=False, debug=False)
    x_d = nc.dram_tensor("x", [H, W], BF16, kind="ExternalInput")
    c_d = nc.dram_tensor("consts", [P, ncols], BF16, kind="ExternalInput")
    out_d = nc.dram_tensor("out", [LEVELS + 1, H, W], BF16,
                           kind="ExternalOutput")

    qcnt = [0]

    def dmae():
        if not cfg["alt_q"]:
            return nc.sync
        qcnt[0] += 1
        return nc.sync if qcnt[0] % 2 else nc.scalar

    with tile.TileContext(nc) as tc:
        with tc.tile_pool(name="sb", bufs=1) as sb, \
             tc.tile_pool(name="wst", bufs=2) as wstage, \
             tc.tile_pool(name="yxp", bufs=4) as yxpool, \
             tc.tile_pool(name="yrp", bufs=4) as yrpool, \
             tc.tile_pool(name="ttp", bufs=4) as ttpool, \
             tc.tile_pool(name="pp", bufs=2) as ppool, \
             tc.tile_pool(name="hlp", bufs=2) as hlpool, \
             tc.tile_pool(name="pf", bufs=(2 if cfg["ph1024"] else 4),
                          space="PSUM") as pf, \
             tc.tile_pool(name="pc", bufs=2, space="PSUM") as pcp, \
             tc.tile_pool(name="pt", bufs=2, space="PSUM") as ptp:

            cr = sb.tile([P, ncols], BF16, tag="cr", name="cr")
            (nc.scalar if cfg["c_act_q"] else nc.sync).dma_start(cr[:], c_d[:])
            ident = cr[:, ident_off:ident_off + P]

            if cfg["warmup"]:
                pw_ = pcp.tile([P, 512], F32, tag="pc", name="pwu")
                for i_w in range(cfg["warmup"]):
                    nc.tensor.matmul(pw_[:],
                                     cr[:, ident_off:ident_off + P],
                                     cr[:, 0:512],
                                     start=(i_w == 0),
                                     stop=(i_w == cfg["warmup"] - 1))
                nc.scalar.copy(
                    sb.tile([P, 512], BF16, tag="wu", name="wuo")[:], pw_[:])

            xb = sb.tile([P, NCH, W], BF16, tag="xb", name="xb")
            if cfg["x_first_small"]:
                xparts = [(0, 1), (1, 1), (2, 1), (3, 1), (4, 2), (6, 2)]
            else:
                ng = cfg["xq"]
                step = NCH // ng
                xparts = [(qv * step, step) for qv in range(ng)]
            for i_x, (c0, ncn) in enumerate(xparts):
                if cfg["x_split"]:
                    eng = nc.sync if i_x < len(xparts) // 2 else nc.scalar
                elif cfg["x_act_q"]:
                    eng = nc.scalar
                else:
                    eng = dmae()
                eng.dma_start(
                    xb[:, c0:c0 + ncn, :],
                    bass.AP(x_d, c0 * P * W,
                            [[W, P], [P * W, ncn], [1, W]]))

            cn = [sb.tile([P, NCH, W], BF16, tag=f"cn{i}", name=f"cn{i}")
                  for i in range(3)]

            for j in range(LEVELS):
                d = DILS[j]
                prev = xb if j == 0 else cn[j - 1]
                cur = cn[j]
                hb = hidx[j]
                wb = widx[j]
                ds_set = cfg["ds"][j]
                si_set = cfg["si"][j]
                hy_set = cfg["hy"][j]
                eb = eidx[j] if cfg["halo"] else None
                p1eng = nc.vector if cfg["p1_dve"][j] else nc.gpsimd

                hl = None
                if cfg["halo"]:
                    # gather the 8-row halos of every chunk into a
                    # 16-partition tile: [0:8]=bottom (from co+1 rows 0-7),
                    # [8:16]=top (from co-1 rows 120-127); chunk 7's top
                    # halo lands at [0:8] of column 8.
                    hl = hlpool.tile([16, NCH + 1, W], BF16, tag="hl",
                                     name="hl")
                    ns_ = cfg["halo_split"]
                    for s0 in range(ns_):
                        lo = s0 * 7 // ns_
                        hi = (s0 + 1) * 7 // ns_
                        if hi <= lo:
                            continue
                        nc.sync.dma_start(
                            hl[8:16, 1 + lo:1 + hi, :],
                            prev[120:128, lo:hi, :])
                        nc.sync.dma_start(
                            hl[0:8, lo:hi, :],
                            prev[0:8, 1 + lo:1 + hi, :])
                    nc.sync.dma_start(hl[0:8, NCH, :],
                                      prev[120:128, NCH - 2, :])

                yxs = {}

                def do_chunk(co, j=j, d=d, prev=prev, cur=cur, hb=hb,
                             wb=wb, ds_set=ds_set, si_set=si_set,
                             hy_set=hy_set, yxs=yxs, eb=eb, hl=hl):
                    if cfg["halo"]:
                        pairs = None
                    else:
                        pairs = sorted((ci, off)
                                       for (o, ci), off in hb.items()
                                       if o == co)

                    def hmm_half(ph, half, out_slice=slice(0, 512)):
                        if cfg["halo"]:
                            nc.tensor.matmul(
                                ph[:, out_slice],
                                cr[:, hb[(co, co)]:hb[(co, co)] + P],
                                prev[:, co, half * 512:(half + 1) * 512],
                                start=True, stop=False)
                            eoff, nr = eb[co]
                            col = NCH if co == NCH - 1 else co
                            nc.tensor.matmul(
                                ph[:, out_slice],
                                cr[0:nr, eoff:eoff + P],
                                hl[0:nr, col,
                                   half * 512:(half + 1) * 512],
                                start=False, stop=True)
                        else:
                            for i, (ci, off) in enumerate(pairs):
                                nc.tensor.matmul(
                                    ph[:, out_slice],
                                    cr[:, off:off + P],
                                    prev[:, ci, half * 512:(half + 1) * 512],
                                    start=(i == 0),
                                    stop=(i == len(pairs) - 1))
                    if co in ds_set:
                        dst = yrpool.tile([P, W], BF16, tag="yr", name="yr")
                        dof = 0
                    else:
                        dst = yxpool.tile([P, WE], BF16, tag="yx", name="yx")
                        dof = MARG
                    if cfg["ph1024"]:
                        ph = pf.tile([P, W], F32, tag="pf", name="ph")
                        for half in range(2):
                            hmm_half(ph, half,
                                     slice(half * 512, (half + 1) * 512))
                        nc.scalar.copy(dst[:, dof:dof + W], ph[:])
                    else:
                        for half in range(2):
                            ph = pf.tile([P, 512], F32, tag="pf", name="ph")
                            hmm_half(ph, half)
                            nc.scalar.copy(
                                dst[:, dof + half * 512:
                                    dof + (half + 1) * 512],
                                ph[:])
                    if co in ds_set:
                        yr = dst
                        tps = ptp.tile([P, W], BF16, tag="pt", name="tps")
                        for q in range(NCH):
                            nc.tensor.transpose(
                                tps[:, q * P:(q + 1) * P],
                                yr[:, q * P:(q + 1) * P], ident)
                        tT = ttpool.tile([P, W], BF16, tag="tT", name="tT")
                        if cfg["tt_split"]:
                            nc.scalar.copy(tT[:, 0:512], tps[:, 0:512])
                            nc.vector.tensor_copy(tT[:, 512:W],
                                                  tps[:, 512:W])
                        else:
                            nc.scalar.copy(tT[:], tps[:])
                        for half in range(2):
                            psc = pcp.tile([P, 512], F32, tag="pc",
                                           name="psc")
                            for qo in range(half * 4, half * 4 + 4):
                                qs = [q for q in range(NCH) if (qo, q) in wb]
                                for i, q in enumerate(qs):
                                    off = wb[(qo, q)]
                                    nc.tensor.matmul(
                                        psc[:, (qo - half * 4) * P:
                                            (qo - half * 4 + 1) * P],
                                        tT[:, q * P:(q + 1) * P],
                                        cr[:, off:off + P],
                                        start=(i == 0),
                                        stop=(i == len(qs) - 1))
                            nc.scalar.copy(
                                cur[:, co, half * 512:(half + 1) * 512],
                                psc[:])
                    else:
                        yx = dst
                        meng = (nc.scalar if (co in si_set or co in hy_set)
                                else nc.gpsimd)
                        (meng.copy if meng is nc.scalar
                         else meng.tensor_copy)(
                            bass.AP(yx.tensor, 0, [[WE, P], [1, MARG]]),
                            bass.AP(yx.tensor, 2 * MARG, [[WE, P], [-1, MARG]]))
                        (meng.copy if meng is nc.scalar
                         else meng.tensor_copy)(
                            bass.AP(yx.tensor, MARG + W, [[WE, P], [1, MARG]]),
                            bass.AP(yx.tensor, MARG + W - 2,
                                    [[WE, P], [-1, MARG]]))
                        if co in si_set:
                            # shifted-identity W-pass: 5 dilated shifts of
                            # yx accumulated on PE with k*I stationaries
                            taps = [(-2 * d, ident_off), (2 * d, ident_off),
                                    (-d, id4_off), (d, id4_off),
                                    (0, id6_off)]
                            for half in range(2):
                                psc = pcp.tile([P, 512], F32, tag="pc",
                                               name="psc")
                                base = MARG + half * 512
                                for i, (off_w, ioff) in enumerate(taps):
                                    nc.tensor.matmul(
                                        psc[:],
                                        cr[:, ioff:ioff + P],
                                        yx[:, base + off_w:
                                           base + off_w + 512],
                                        start=(i == 0),
                                        stop=(i == len(taps) - 1))
                                nc.scalar.copy(
                                    cur[:, co, half * 512:(half + 1) * 512],
                                    psc[:])
                        elif co in hy_set:
                            # PE (1,2,1) pass into p2, DVE finishes with
                            # two (1,1) passes: (1,2,1)*(1,2,1) = 5-tap
                            p2x = yxpool.tile([P, WE], BF16, tag="p2x",
                                              name="p2x")
                            taps3 = [(-d, ident_off), (d, ident_off),
                                     (0, id2_off)]
                            for half in range(2):
                                psc = pcp.tile([P, 512], F32, tag="pc",
                                               name="psc")
                                base = MARG + half * 512
                                for i, (off_w, ioff) in enumerate(taps3):
                                    nc.tensor.matmul(
                                        psc[:],
                                        cr[:, ioff:ioff + P],
                                        yx[:, base + off_w:
                                           base + off_w + 512],
                                        start=(i == 0),
                                        stop=(i == len(taps3) - 1))
                                nc.scalar.copy(
                                    p2x[:, MARG + half * 512:
                                        MARG + (half + 1) * 512],
                                    psc[:])
                            m2 = (nc.vector if cfg["hy_marg_dve"]
                                  else nc.gpsimd)
                            m2.tensor_copy(
                                bass.AP(p2x.tensor, MARG - d,
                                        [[WE, P], [1, d]]),
                                bass.AP(p2x.tensor, MARG + d,
                                        [[WE, P], [-1, d]]))
                            m2.tensor_copy(
                                bass.AP(p2x.tensor, MARG + W,
                                        [[WE, P], [1, d]]),
                                bass.AP(p2x.tensor, MARG + W - 2,
                                        [[WE, P], [-1, d]]))
                            q1t = ppool.tile([P, W + MARG], BF16, tag="q1",
                                             name="q1")
                            nc.vector.tensor_add(
                                q1t[:, 0:W + d],
                                p2x[:, MARG - d:MARG - d + W + d],
                                p2x[:, MARG:MARG + W + d])
                            nc.vector.tensor_add(
                                cur[:, co, :], q1t[:, 0:W],
                                q1t[:, d:d + W])
                        else:
                            yxs[co] = yx

                rt_seen = [0]

                def do_route(co, j=j, d=d, cur=cur, yxs=yxs, p1eng=p1eng,
                             rt_seen=rt_seen):
                    if cfg["p1k"] is not None:
                        p1eng = (nc.vector if rt_seen[0] < cfg["p1k"][j]
                                 else nc.gpsimd)
                        rt_seen[0] += 1
                    yx = yxs[co]
                    w1_ = WE - d
                    w2_ = WE - 2 * d
                    w3_ = WE - 3 * d
                    p1 = ppool.tile([P, WE], BF16, tag="p1", name="p1")
                    p2 = ppool.tile([P, WE], BF16, tag="p2", name="p2")
                    p1eng.tensor_add(p1[:, 0:w1_], yx[:, 0:w1_],
                                     yx[:, d:d + w1_])
                    nc.vector.tensor_add(p2[:, 0:w2_], p1[:, 0:w2_],
                                         p1[:, d:d + w2_])
                    nc.vector.tensor_add(p1[:, 0:w3_], p2[:, 0:w3_],
                                         p2[:, d:d + w3_])
                    off = MARG - 2 * d
                    nc.vector.tensor_add(cur[:, co, :], p1[:, off:off + W],
                                         p1[:, off + d:off + d + W])

                def do_out(qv, j=j, prev=prev, cur=cur):
                    fine = cfg["out8_l2"] in (True, "all") and (cfg["out8_l2"] == "all" or j == LEVELS - 1)
                    wst = wstage.tile([P, 2, W], BF16, tag="wst", name="wst")
                    for i_, co in enumerate(range(qv * 2, qv * 2 + 2)):
                        eng = (nc.gpsimd if co in cfg["sub_pool"][j]
                               else nc.vector)
                        eng.tensor_sub(wst[:, i_, :], prev[:, co, :],
                                       cur[:, co, :])
                        if fine:
                            dmae().dma_start(
                                bass.AP(out_d, j * H * W + co * P * W,
                                        [[W, P], [1, W]]),
                                wst[:, i_, :])
                    if not fine:
                        dmae().dma_start(
                            bass.AP(out_d, j * H * W + qv * 2 * P * W,
                                    [[W, P], [P * W, 2], [1, W]]),
                            wst[:])
                    if j == LEVELS - 1:
                        if fine:
                            for co in range(qv * 2, qv * 2 + 2):
                                (nc.scalar if cfg["c3_act_q"]
                                 else dmae()).dma_start(
                                    bass.AP(out_d, 3 * H * W + co * P * W,
                                            [[W, P], [1, W]]),
                                    cur[:, co, :])
                        else:
                            (nc.scalar if cfg["c3_act_q"]
                             else dmae()).dma_start(
                                bass.AP(out_d, 3 * H * W + qv * 2 * P * W,
                                        [[W, P], [P * W, 2], [1, W]]),
                                cur[:, qv * 2:(qv + 1) * 2, :])

                nonroute = set(ds_set) | set(si_set) | set(hy_set)
                if cfg["qorder"]:
                    for qv in range(4):
                        for co in (qv * 2, qv * 2 + 1):
                            do_chunk(co)
                        for co in (qv * 2, qv * 2 + 1):
                            if co not in nonroute:
                                do_route(co)
                        do_out(qv)
                else:
                    for co in range(NCH):
                        do_chunk(co)
                    for co in range(NCH):
                        if co not in nonroute:
                            do_route(co)
                    for qv in range(4):
                        do_out(qv)

    nc.compile()
    return nc, consts_np


_CACHE = {}


def _get_program():
    if "prog" not in _CACHE:
        _CACHE["prog"] = _build_program()
    return _CACHE["prog"]


def kernel(x, _trace=False, _trace_kwargs=None):
    """x: [8, 1024, 1024] float32 -> [8, 4, 1024, 1024] float32."""
    x = np.asarray(x)
    assert x.shape == (B, H, W) and x.dtype == np.float32
    nc, consts_np = _get_program()
    xb = x.astype(ml_dtypes.bfloat16)
    in_maps = [{"x": np.ascontiguousarray(xb[b]), "consts": consts_np}
               for b in range(B)]
    kw = {}
    if _trace:
        kw = dict(trace=True, **(_trace_kwargs or {}))
    res = run_bass_kernel_spmd(nc, in_maps, core_ids=list(range(B)), **kw)
    out = np.stack([np.asarray(r["out"]).astype(np.float32)
                    for r in res.results], axis=0)
    if _trace:
        return out, res
    return out


# revision 8
# speedup vs baseline: 1.0004x; 1.0004x over previous
"""Trainium2 kernel v2 for nn_B3SplineUWT: 3-level B3-spline UWT,
data-parallel over 8 NeuronCores, bf16 data path.

kernel(x: [8,1024,1024] f32) -> [8,4,1024,1024] f32  (w1,w2,w3,c3)

Per core, per level (chained c_{j+1} = A_w A_h c_j):
  - H-conv: PE banded matmuls, lhsT = A_d/16 blocks (bf16-exact).
  - W-conv, "ds" chunks: PE transpose -> evac -> data-stationary
    matmul (moving = 16*A_d blocks) -> ACT evac. No transpose-back.
  - W-conv, "route" chunks: 4 binomial (1,1) passes on DVE/Pool
    (taps 1,4,6,4,1 = (1,1)^4; /16 pre-folded into H consts).
  - w_j = prev - cur: bf16 TT; outputs DMA'd as bf16, host upcasts.
"""
import numpy as np
import ml_dtypes

import concourse.bacc as bacc
import concourse.bass as bass
import concourse.mybir as mybir
import concourse.tile as tile
from concourse.bass_utils import run_bass_kernel_spmd

F32 = mybir.dt.float32
BF16 = mybir.dt.bfloat16
ADD = mybir.AluOpType.add
SUB = mybir.AluOpType.subtract
COPY = mybir.ActivationFunctionType.Copy

B = 8
H = 1024
W = 1024
P = 128
NCH = H // P
LEVELS = 3
DILS = (1, 2, 4)
MARG = 8
WE = W + 2 * MARG

TAPS = {0: 3.0 / 8, 1: 1.0 / 4, 2: 1.0 / 16}

DEFAULT_CFG = {
    "ds": ((2,), (0,), (1,)),          # per-level ds chunk sets
    "si": ((6,), (3, 6), (3, 6, 7)),   # per-level shifted-identity chunks
    "hy": ((), (), ()),                # hybrid PE(1,2,1)+DVE(1,2,1) chunks
    "hy_marg_dve": False,              # hybrid p2 margins on DVE not Pool
    "tt_split": False,                 # tT evac: half ACT, half DVE
    "p1_dve": (False, False, False),   # per-level: p1 pass on DVE not Pool
    "alt_q": False,                    # alternate HWDGE queues SP/ACT
    "sub_pool": ((), (), ()),          # per-level chunks whose sub -> Pool
    "xq": 4,                           # number of input DMAs
    "x_act_q": False,                  # issue input DMAs from ACT queue
    "x_split": False,                  # input DMAs split across both rings
    "ph1024": False,                   # single 1024-wide H psum + one evac
    "qorder": True,                    # quarter-interleaved emission
    "p1k": None,                       # per-level: first-k route p1 on DVE
    "c3_act_q": False,                 # c3 DMAs on ACT HWDGE ring
    "x_first_small": True,             # input DMA sizes 1,1,2,4 chunks
    "warmup": 0,                       # dummy PE MMs after consts load
    "out8_l2": True,                   # per-chunk output DMAs on last level
    "c_act_q": False,                  # consts DMA on ACT HWDGE ring
    "halo": False,                     # DMA-gathered halo H-conv (16 MMs)
    "halo_split": 1,                   # pieces per halo DMA
}


def _reflect(i, n):
    if i < 0:
        return -i
    if i >= n:
        return 2 * (n - 1) - i
    return i


def _build_full(d, scale):
    full = np.zeros((H, H), np.float32)
    for r in range(H):
        for o in (-2 * d, -d, 0, d, 2 * d):
            full[_reflect(r + o, H), r] += TAPS[abs(o) // d] * scale
    return full


def _pack_consts(with_id2=False, with_halo=False):
    mats, seen = [], {}
    hidx, widx = [], []
    eidx = []   # per level: (interior16_off, co0_off, co7_off)

    def intern(blk):
        key = blk.tobytes()
        if key not in seen:
            seen[key] = len(mats) * P
            mats.append(blk)
        return seen[key]

    for d in DILS:
        fh = _build_full(d, 1.0 / 16)
        idx = {}
        for co in range(NCH):
            for ci in range(NCH):
                blk = np.ascontiguousarray(fh[ci * P:(ci + 1) * P,
                                              co * P:(co + 1) * P])
                if np.any(blk != 0):
                    idx[(co, ci)] = intern(blk.astype(ml_dtypes.bfloat16))
        hidx.append(idx)
    for d in (DILS if with_halo else ()):
        # halo stationaries: per chunk a [16,P] block (rows 0-7 =
        # bottom-halo taps from chunk co+1 rows 0-7; rows 8-15 = top-halo
        # taps from chunk co-1 rows 120-127), padded into [128,P] slots.
        # Chunk 0/7 get 8-row one-sided blocks.
        fh = _build_full(d, 1.0 / 16)
        lvl = {}
        for co in range(NCH):
            blk = np.zeros((P, P), np.float32)
            nr = 0
            if co < NCH - 1:
                blk[0:8, :] = fh[(co + 1) * P:(co + 1) * P + 8,
                                 co * P:(co + 1) * P]
                nr = 8
            if co > 0:
                if co == NCH - 1:
                    # top-only at rows 0-7 (read via hl column 8, base 0)
                    blk[0:8, :] = fh[co * P - 8:co * P, co * P:(co + 1) * P]
                    nr = 8
                else:
                    blk[8:16, :] = fh[co * P - 8:co * P,
                                      co * P:(co + 1) * P]
                    nr = 16
            lvl[co] = (intern(blk.astype(ml_dtypes.bfloat16)), nr)
        eidx.append(lvl)
    for d in DILS:
        fw = _build_full(d, 16.0)
        idx = {}
        for co in range(NCH):
            for ci in range(NCH):
                blk = np.ascontiguousarray(fw[ci * P:(ci + 1) * P,
                                              co * P:(co + 1) * P])
                if np.any(blk != 0):
                    idx[(co, ci)] = intern(blk.astype(ml_dtypes.bfloat16))
        widx.append(idx)
    ident_off = len(mats) * P
    mats.append(np.eye(P, dtype=ml_dtypes.bfloat16))
    id4_off = len(mats) * P
    mats.append((4 * np.eye(P)).astype(ml_dtypes.bfloat16))
    id6_off = len(mats) * P
    mats.append((6 * np.eye(P)).astype(ml_dtypes.bfloat16))
    id2_off = None
    if with_id2:
        id2_off = len(mats) * P
        mats.append((2 * np.eye(P)).astype(ml_dtypes.bfloat16))
    packed = np.ascontiguousarray(np.concatenate(mats, axis=1))
    return packed, hidx, widx, eidx, (ident_off, id4_off, id6_off, id2_off)


def _build_program(cfg=None):
    cfg = dict(DEFAULT_CFG, **(cfg or {}))
    consts_np, hidx, widx, eidx, (ident_off, id4_off, id6_off, id2_off) = \
        _pack_consts(with_id2=any(cfg["hy"]), with_halo=cfg["halo"])
    ncols = consts_np.shape[1]

    nc = bacc.Bacc("TRN2", target_bir_lowering=False, debug=False)
    x_d = nc.dram_tensor("x", [H, W], BF16, kind="ExternalInput")
    c_d = nc.dram_tensor("consts", [P, ncols], BF16, kind="ExternalInput")
    out_d = nc.dram_tensor("out", [LEVELS + 1, H, W], BF16,
                           kind="ExternalOutput")

    qcnt = [0]

    def dmae():
        if not cfg["alt_q"]:
            return nc.sync
        qcnt[0] += 1
        return nc.sync if qcnt[0] % 2 else nc.scalar

    with tile.TileContext(nc) as tc:
        with tc.tile_pool(name="sb", bufs=1) as sb, \
             tc.tile_pool(name="wst", bufs=2) as wstage, \
             tc.tile_pool(name="yxp", bufs=4) as yxpool, \
             tc.tile_pool(name="yrp", bufs=4) as yrpool, \
             tc.tile_pool(name="ttp", bufs=4) as ttpool, \
             tc.tile_pool(name="pp", bufs=2) as ppool, \
             tc.tile_pool(name="hlp", bufs=2) as hlpool, \
             tc.tile_pool(name="pf", bufs=(2 if cfg["ph1024"] else 4),
                          space="PSUM") as pf, \
             tc.tile_pool(name="pc", bufs=2, space="PSUM") as pcp, \
             tc.tile_pool(name="pt", bufs=2, space="PSUM") as ptp:

            cr = sb.tile([P, ncols], BF16, tag="cr", name="cr")
            (nc.scalar if cfg["c_act_q"] else nc.sync).dma_start(cr[:], c_d[:])
            ident = cr[:, ident_off:ident_off + P]

            if cfg["warmup"]:
                pw_ = pcp.tile([P, 512], F32, tag="pc", name="pwu")
                for i_w in range(cfg["warmup"]):
                    nc.tensor.matmul(pw_[:],
                                     cr[:, ident_off:ident_off + P],
                                     cr[:, 0:512],
                                     start=(i_w == 0),
                                     stop=(i_w == cfg["warmup"] - 1))
                nc.scalar.copy(
                    sb.tile([P, 512], BF16, tag="wu", name="wuo")[:], pw_[:])

            xb = sb.tile([P, NCH, W], BF16, tag="xb", name="xb")
            if cfg["x_first_small"]:
                xparts = [(0, 1), (1, 1), (2, 1), (3, 1), (4, 2), (6, 2)]
            else:
                ng = cfg["xq"]
                step = NCH // ng
                xparts = [(qv * step, step) for qv in range(ng)]
            for i_x, (c0, ncn) in enumerate(xparts):
                if cfg["x_split"]:
                    eng = nc.sync if i_x < len(xparts) // 2 else nc.scalar
                elif cfg["x_act_q"]:
                    eng = nc.scalar
                else:
                    eng = dmae()
                eng.dma_start(
                    xb[:, c0:c0 + ncn, :],
                    bass.AP(x_d, c0 * P * W,
                            [[W, P], [P * W, ncn], [1, W]]))

            cn = [sb.tile([P, NCH, W], BF16, tag=f"cn{i}", name=f"cn{i}")
                  for i in range(3)]

            for j in range(LEVELS):
                d = DILS[j]
                prev = xb if j == 0 else cn[j - 1]
                cur = cn[j]
                hb = hidx[j]
                wb = widx[j]
                ds_set = cfg["ds"][j]
                si_set = cfg["si"][j]
                hy_set = cfg["hy"][j]
                eb = eidx[j] if cfg["halo"] else None
                p1eng = nc.vector if cfg["p1_dve"][j] else nc.gpsimd

                hl = None
                if cfg["halo"]:
                    # gather the 8-row halos of every chunk into a
                    # 16-partition tile: [0:8]=bottom (from co+1 rows 0-7),
                    # [8:16]=top (from co-1 rows 120-127); chunk 7's top
                    # halo lands at [0:8] of column 8.
                    hl = hlpool.tile([16, NCH + 1, W], BF16, tag="hl",
                                     name="hl")
                    ns_ = cfg["halo_split"]
                    for s0 in range(ns_):
                        lo = s0 * 7 // ns_
                        hi = (s0 + 1) * 7 // ns_
                        if hi <= lo:
                            continue
                        nc.sync.dma_start(
                            hl[8:16, 1 + lo:1 + hi, :],
                            prev[120:128, lo:hi, :])
                        nc.sync.dma_start(
                            hl[0:8, lo:hi, :],
                            prev[0:8, 1 + lo:1 + hi, :])
                    nc.sync.dma_start(hl[0:8, NCH, :],
                                      prev[120:128, NCH - 2, :])

                yxs = {}

                def do_chunk(co, j=j, d=d, prev=prev, cur=cur, hb=hb,
                             wb=wb, ds_set=ds_set, si_set=si_set,
                             hy_set=hy_set, yxs=yxs, eb=eb, hl=hl):
                    if cfg["halo"]:
                        pairs = None
                    else:
                        pairs = sorted((ci, off)
                                       for (o, ci), off in hb.items()
                                       if o == co)

                    def hmm_half(ph, half, out_slice=slice(0, 512)):
                        if cfg["halo"]:
                            nc.tensor.matmul(
                                ph[:, out_slice],
                                cr[:, hb[(co, co)]:hb[(co, co)] + P],
                                prev[:, co, half * 512:(half + 1) * 512],
                                start=True, stop=False)
                            eoff, nr = eb[co]
                            col = NCH if co == NCH - 1 else co
                            nc.tensor.matmul(
                                ph[:, out_slice],
                                cr[0:nr, eoff:eoff + P],
                                hl[0:nr, col,
                                   half * 512:(half + 1) * 512],
                                start=False, stop=True)
                        else:
                            for i, (ci, off) in enumerate(pairs):
                                nc.tensor.matmul(
                                    ph[:, out_slice],
                                    cr[:, off:off + P],
                                    prev[:, ci, half * 512:(half + 1) * 512],
                                    start=(i == 0),
                                    stop=(i == len(pairs) - 1))
                    if co in ds_set:
                        dst = yrpool.tile([P, W], BF16, tag="yr", name="yr")
                        dof = 0
                    else:
                        dst = yxpool.tile([P, WE], BF16, tag="yx", name="yx")
                        dof = MARG
                    if cfg["ph1024"]:
                        ph = pf.tile([P, W], F32, tag="pf", name="ph")
                        for half in range(2):
                            hmm_half(ph, half,
                                     slice(half * 512, (half + 1) * 512))
                        nc.scalar.copy(dst[:, dof:dof + W], ph[:])
                    else:
                        for half in range(2):
                            ph = pf.tile([P, 512], F32, tag="pf", name="ph")
                            hmm_half(ph, half)
                            nc.scalar.copy(
                                dst[:, dof + half * 512:
                                    dof + (half + 1) * 512],
                                ph[:])
                    if co in ds_set:
                        yr = dst
                        tps = ptp.tile([P, W], BF16, tag="pt", name="tps")
                        for q in range(NCH):
                            nc.tensor.transpose(
                                tps[:, q * P:(q + 1) * P],
                                yr[:, q * P:(q + 1) * P], ident)
                        tT = ttpool.tile([P, W], BF16, tag="tT", name="tT")
                        if cfg["tt_split"]:
                            nc.scalar.copy(tT[:, 0:512], tps[:, 0:512])
                            nc.vector.tensor_copy(tT[:, 512:W],
                                                  tps[:, 512:W])
                        else:
                            nc.scalar.copy(tT[:], tps[:])
                        for half in range(2):
                            psc = pcp.tile([P, 512], F32, tag="pc",
                                           name="psc")
                            for qo in range(half * 4, half * 4 + 4):
                                qs = [q for q in range(NCH) if (qo, q) in wb]
                                for i, q in enumerate(qs):
                                    off = wb[(qo, q)]
                                    nc.tensor.matmul(
                                        psc[:, (qo - half * 4) * P:
                                            (qo - half * 4 + 1) * P],
                                        tT[:, q * P:(q + 1) * P],
                                        cr[:, off:off + P],
                                        start=(i == 0),
                                        stop=(i == len(qs) - 1))
                            nc.scalar.copy(
                                cur[:, co, half * 512:(half + 1) * 512],
                                psc[:])
                    else:
                        yx = dst
                        meng = (nc.scalar if (co in si_set or co in hy_set)
                                else nc.gpsimd)
                        (meng.copy if meng is nc.scalar
                         else meng.tensor_copy)(
                            bass.AP(yx.tensor, 0, [[WE, P], [1, MARG]]),
                            bass.AP(yx.tensor, 2 * MARG, [[WE, P], [-1, MARG]]))
                        (meng.copy if meng is nc.scalar
                         else meng.tensor_copy)(
                            bass.AP(yx.tensor, MARG + W, [[WE, P], [1, MARG]]),
                            bass.AP(yx.tensor, MARG + W - 2,
                                    [[WE, P], [-1, MARG]]))
                        if co in si_set:
                            # shifted-identity W-pass: 5 dilated shifts of
                            # yx accumulated on PE with k*I stationaries
                            taps = [(-2 * d, ident_off), (2 * d, ident_off),
                                    (-d, id4_off), (d, id4_off),
                                    (0, id6_off)]
                            for half in range(2):
                                psc = pcp.tile([P, 512], F32, tag="pc",
                                               name="psc")
                                base = MARG + half * 512
                                for i, (off_w, ioff) in enumerate(taps):
                                    nc.tensor.matmul(
                                        psc[:],
                                        cr[:, ioff:ioff + P],
                                        yx[:, base + off_w:
                                           base + off_w + 512],
                                        start=(i == 0),
                                        stop=(i == len(taps) - 1))
                                nc.scalar.copy(
                                    cur[:, co, half * 512:(half + 1) * 512],
                                    psc[:])
                        elif co in hy_set:
                            # PE (1,2,1) pass into p2, DVE finishes with
                            # two (1,1) passes: (1,2,1)*(1,2,1) = 5-tap
                            p2x = yxpool.tile([P, WE], BF16, tag="p2x",
                                              name="p2x")
                            taps3 = [(-d, ident_off), (d, ident_off),
                                     (0, id2_off)]
                            for half in range(2):
                                psc = pcp.tile([P, 512], F32, tag="pc",
                                               name="psc")
                                base = MARG + half * 512
                                for i, (off_w, ioff) in enumerate(taps3):
                                    nc.tensor.matmul(
                                        psc[:],
                                        cr[:, ioff:ioff + P],
                                        yx[:, base + off_w:
                                           base + off_w + 512],
                                        start=(i == 0),
                                        stop=(i == len(taps3) - 1))
                                nc.scalar.copy(
                                    p2x[:, MARG + half * 512:
                                        MARG + (half + 1) * 512],
                                    psc[:])
                            m2 = (nc.vector if cfg["hy_marg_dve"]
                                  else nc.gpsimd)
                            m2.tensor_copy(
                                bass.AP(p2x.tensor, MARG - d,
                                        [[WE, P], [1, d]]),
                                bass.AP(p2x.tensor, MARG + d,
                                        [[WE, P], [-1, d]]))
                            m2.tensor_copy(
                                bass.AP(p2x.tensor, MARG + W,
                                        [[WE, P], [1, d]]),
                                bass.AP(p2x.tensor, MARG + W - 2,
                                        [[WE, P], [-1, d]]))
                            q1t = ppool.tile([P, W + MARG], BF16, tag="q1",
                                             name="q1")
                            nc.vector.tensor_add(
                                q1t[:, 0:W + d],
                                p2x[:, MARG - d:MARG - d + W + d],
                                p2x[:, MARG:MARG + W + d])
                            nc.vector.tensor_add(
                                cur[:, co, :], q1t[:, 0:W],
                                q1t[:, d:d + W])
                        else:
                            yxs[co] = yx

                rt_seen = [0]

                def do_route(co, j=j, d=d, cur=cur, yxs=yxs, p1eng=p1eng,
                             rt_seen=rt_seen):
                    if cfg["p1k"] is not None:
                        p1eng = (nc.vector if rt_seen[0] < cfg["p1k"][j]
                                 else nc.gpsimd)
                        rt_seen[0] += 1
                    yx = yxs[co]
                    w1_ = WE - d
                    w2_ = WE - 2 * d
                    w3_ = WE - 3 * d
                    p1 = ppool.tile([P, WE], BF16, tag="p1", name="p1")
                    p2 = ppool.tile([P, WE], BF16, tag="p2", name="p2")
                    p1eng.tensor_add(p1[:, 0:w1_], yx[:, 0:w1_],
                                     yx[:, d:d + w1_])
                    nc.vector.tensor_add(p2[:, 0:w2_], p1[:, 0:w2_],
                                         p1[:, d:d + w2_])
                    nc.vector.tensor_add(p1[:, 0:w3_], p2[:, 0:w3_],
                                         p2[:, d:d + w3_])
                    off = MARG - 2 * d
                    nc.vector.tensor_add(cur[:, co, :], p1[:, off:off + W],
                                         p1[:, off + d:off + d + W])

                def do_out(qv, j=j, prev=prev, cur=cur):
                    fine = cfg["out8_l2"] in (True, "all") and (cfg["out8_l2"] == "all" or j == LEVELS - 1)
                    wst = wstage.tile([P, 2, W], BF16, tag="wst", name="wst")
                    for i_, co in enumerate(range(qv * 2, qv * 2 + 2)):
                        eng = (nc.gpsimd if co in cfg["sub_pool"][j]
                               else nc.vector)
                        eng.tensor_sub(wst[:, i_, :], prev[:, co, :],
                                       cur[:, co, :])
                        if fine:
                            dmae().dma_start(
                                bass.AP(out_d, j * H * W + co * P * W,
                                        [[W, P], [1, W]]),
                                wst[:, i_, :])
                    if not fine:
                        dmae().dma_start(
                            bass.AP(out_d, j * H * W + qv * 2 * P * W,
                                    [[W, P], [P * W, 2], [1, W]]),
                            wst[:])
                    if j == LEVELS - 1:
                        if fine:
                            for co in range(qv * 2, qv * 2 + 2):
                                (nc.scalar if cfg["c3_act_q"]
                                 else dmae()).dma_start(
                                    bass.AP(out_d, 3 * H * W + co * P * W,
                                            [[W, P], [1, W]]),
                                    cur[:, co, :])
                        else:
                            (nc.scalar if cfg["c3_act_q"]
                             else dmae()).dma_start(
                                bass.AP(out_d, 3 * H * W + qv * 2 * P * W,
                                        [[W, P], [P * W, 2], [1, W]]),
                                cur[:, qv * 2:(qv + 1) * 2, :])

                nonroute = set(ds_set) | set(si_set) | set(hy_set)
                if cfg["qorder"]:
                    for qv in range(4):
                        for co in (qv * 2, qv * 2 + 1):
                            do_chunk(co)
                        for co in (qv * 2, qv * 2 + 1):
                            if co not in nonroute:
                                do_route(co)
                        do_out(qv)
                else:
                    for co in range(NCH):
                        do_chunk(co)
                    for co in range(NCH):
                        if co not in nonroute:
                            do_route(co)
                    for qv in range(4):
                        do_out(qv)

    nc.compile()
    return nc, consts_np


_CACHE = {}


def _get_program():
    if "prog" not in _CACHE:
        _CACHE["prog"] = _build_program()
    return _CACHE["prog"]


def kernel(x, _trace=False, _trace_kwargs=None):
    """x: [8, 1024, 1024] float32 -> [8, 4, 1024, 1024] float32."""
    x = np.asarray(x)
    assert x.shape == (B, H, W) and x.dtype == np.float32
    nc, consts_np = _get_program()
    xb = x.astype(ml_dtypes.bfloat16)
    in_maps = [{"x": np.ascontiguousarray(xb[b]), "consts": consts_np}
               for b in range(B)]
    kw = {}
    if _trace:
        kw = dict(trace=True, **(_trace_kwargs or {}))
    res = run_bass_kernel_spmd(nc, in_maps, core_ids=list(range(B)), **kw)
    out = np.stack([np.asarray(r["out"]).astype(np.float32)
                    for r in res.results], axis=0)
    if _trace:
        return out, res
    return out
